# revision 6
# baseline (speedup 1.0000x reference)
import sys

for _p in ("/opt/trn_rl_repo", "/root/.axon_site/_ro/trn_rl_repo"):
    if _p not in sys.path:
        sys.path.insert(0, _p)

import numpy as np
import ml_dtypes

# nn_LocalConvolution: x [4,256,64,64] f32, weight [4,1,16,49,64,64] f32,
# K=7, pad=3, stride=1, dil=1, heads=1, wc=16, G=16.
# out[b, g*16+cc, y, x] = sum_{kh,kw} x_pad[b, g*16+cc, y+kh-3, x+kw-3]
#                                     * w[b,0,cc,kh*7+kw,y,x]
#
# Shard: core = b*2 + h (batch x H-half). Partition p = cc*8 + ysub covers
# output rows yloc = ysub*4 + y4 (y4 in 0..3) of the half => weights need NO
# cross-partition replication (8x less weight DMA than channel-major).
# Per-partition free layout is (g, y4, x) so every engine AP fits the ISA's
# 3-free-dim limit (weight broadcasts over g with stride 0).
B, C, H, W = 4, 256, 64, 64
WC, KK, K, PAD = 16, 49, 7, 3
G = C // WC
NCORES = 8
PART = 128
HHALF = H // 2  # 32 rows per core
NY = 4          # output rows per partition
NSUB = 8        # ysub count
HALO = NY + 2 * PAD  # 10 input rows per partition window
XC = W + 2 * PAD     # 70 padded cols
FREE = G * NY * W    # 4096 per-partition output elems (g, y4, x)
BANK = 512
NBANK = FREE // BANK  # 8 psum banks; bank j = g in {2j, 2j+1}
# x row chunks (overlap-free coverage of tap windows):
# P = rows 0..4 (kh 0,1), Q = rows 2..9 (kh 2..6)
PROWS, QROW0, QROWS = 5, 2, 8

N_POOL = 9  # singles offloaded to gpsimd: kh 0..3 x kw {1,3} + kh 0..1 kw5

_BF16 = ml_dtypes.bfloat16
_cache = {}


def _build(n_pool=N_POOL):
    import concourse.bacc as bacc
    import concourse.mybir as mybir
    import concourse.tile as tile

    nc = bacc.Bacc(None, target_bir_lowering=False)
    bf = mybir.dt.bfloat16
    f32 = mybir.dt.float32

    xaPA_d = nc.dram_tensor("xaPA", (PART, PROWS * 8 * XC), bf, kind="ExternalInput")
    xaPB_d = nc.dram_tensor("xaPB", (PART, PROWS * 8 * XC), bf, kind="ExternalInput")
    xaQ_d = nc.dram_tensor("xaQ", (PART, G * QROWS * XC), bf, kind="ExternalInput")
    xbP_d = nc.dram_tensor("xbP", (PART, G * PROWS * XC), bf, kind="ExternalInput")
    xbQ_d = nc.dram_tensor("xbQ", (PART, G * QROWS * XC), bf, kind="ExternalInput")
    w_d = nc.dram_tensor("wr", (PART, KK, NY, W), bf, kind="ExternalInput")
    id_d = nc.dram_tensor("ident", (PART, PART), bf, kind="ExternalInput")
    out_d = nc.dram_tensor("out", (PART, FREE), f32, kind="ExternalOutput")

    # tap schedule (each group = one tap):
    #  - DVE fulls: evens (kw 0,2,4,6) + odds (kw 1,3) kh>=4
    #  - gpsimd: kh 0..3 x kw {1,3} + kh0 kw5
    #  - tail: kw5 kh 1..6 split into g-half TTs (banks 0-3 finish and start
    #    streaming out while banks 4-7 still accumulate)
    pool_groups = [(0, 0, 0), (0, 0, 1), (0, 2, 0), (0, 2, 1)]
    pool_groups += [(kh, kw, None) for kh in range(3) for kw in (1, 3)]
    pool_groups += [(3, 1, None)]
    dve_halves = [(0, 4, 0), (0, 6, 0), (0, 4, 1), (0, 6, 1)]
    dve_halves += [(1, kw, g) for g in (0, 1) for kw in (0, 2, 4, 6)]
    dve_fulls = [(kh, kw) for kh in range(2, K) for kw in (0, 2, 4, 6)]
    dve_fulls += [(3, 3)] + [(kh, kw) for kh in range(4, K) for kw in (1, 3)]
    dve_fulls += [(0, 5)]
    tails = [(kh, 5) for kh in range(1, K)]
    n_head = len(dve_halves)
    q0_stop = n_head + len(dve_fulls) + len(tails) - 1
    pool_mm_pos = {4: 0, 7: 1, 11: 2, 13: 3, 17: 4, 21: 5, 25: 6, 29: 7,
                   32: 8, 36: 9, 41: 10}

    with tile.TileContext(nc) as tc:
        with (
            tc.tile_pool(name="xpool", bufs=1) as xpool,
            tc.tile_pool(name="cpool", bufs=1) as cpool,
            tc.tile_pool(name="wpool", bufs=1) as wpool,
            tc.tile_pool(name="tpool", bufs=6) as tpool,
            tc.tile_pool(name="hpool", bufs=6) as hpool,
            tc.tile_pool(name="gpool", bufs=2) as gpool,
            tc.tile_pool(name="opool", bufs=1) as opool,
            tc.tile_pool(name="psum", bufs=1, space="PSUM") as ppool,
        ):
            id_t = cpool.tile([PART, PART], bf, tag="id")
            nc.sync.dma_start(id_t[:], id_d[:])
            w_t = [
                wpool.tile([PART, K, NY, W], bf, name=f"w{kh}", tag=f"w{kh}")
                for kh in range(K)
            ]
            xaPA_t = xpool.tile([PART, PROWS, 8, XC], bf, tag="xaPA")
            xaPB_t = xpool.tile([PART, PROWS, 8, XC], bf, tag="xaPB")
            xbP_t = xpool.tile([PART, G, PROWS, XC], bf, tag="xbP")
            xaQ_t = xpool.tile([PART, G, QROWS, XC], bf, tag="xaQ")
            xbQ_t = xpool.tile([PART, G, QROWS, XC], bf, tag="xbQ")
            nc.sync.dma_start(w_t[0][:, 0:K:2, :, :], w_d[:, 0:K:2, :, :])
            nc.sync.dma_start(xaPA_t[:], xaPA_d[:])
            nc.sync.dma_start(xaPB_t[:], xaPB_d[:])
            nc.sync.dma_start(w_t[0][:, 1:K:2, :, :], w_d[:, 1:K:2, :, :])
            nc.sync.dma_start(w_t[1][:], w_d[:, K : 2 * K, :, :])
            nc.sync.dma_start(w_t[2][:], w_d[:, 2 * K : 3 * K, :, :])
            nc.sync.dma_start(xaQ_t[:], xaQ_d[:])
            nc.sync.dma_start(xbP_t[:], xbP_d[:])
            for kh in range(3, K):
                nc.sync.dma_start(w_t[kh][:], w_d[:, kh * K : (kh + 1) * K, :, :])
            nc.sync.dma_start(xbQ_t[:], xbQ_d[:])

            acc = [
                ppool.tile([PART, BANK], f32, name=f"ps{j}", tag=f"ps{j}")
                for j in range(NBANK)
            ]
            ost = [
                opool.tile([PART, 2 * BANK], f32, name=f"ost{q}", tag=f"ost{q}")
                for q in range(4)
            ]

            # keep the PE p-state ramped while input DMAs run: cheap dummy
            # matmuls into bank 0 (cleared by the first real start=True mm)
            for _ in range(140):
                nc.tensor.matmul(
                    acc[0][:, 0:PART], id_t[:], id_t[:], start=True, stop=False
                )

            def src_ap(kh, kw, g0=0, ng=G):
                odd = kw % 2
                if kh <= 1 and not odd:
                    t = xaPA_t if g0 < 8 else xaPB_t
                    a = t[:, kh : kh + NY, g0 % 8 : g0 % 8 + min(ng, 8), :][
                        :, :, :, kw : kw + W
                    ]
                    return a.transpose([0, 2, 1, 3])
                if kh <= 1:
                    t, r0 = xbP_t, kh
                else:
                    t, r0 = (xbQ_t if odd else xaQ_t), kh - QROW0
                col0 = kw + odd
                return t[:, g0 : g0 + ng, r0 : r0 + NY, col0 : col0 + W]

            def w_ap(kh, kw, ng=G):
                a = w_t[kh][:, kw, :, :].unsqueeze(1)
                return a.broadcast_to((PART, ng, NY, W))

            def emit_mms(tmp, first, banks=range(NBANK), stop=False):
                for bi, j in enumerate(banks):
                    nc.tensor.matmul(
                        acc[j][:],
                        id_t[:],
                        tmp[:, 2 * bi : 2 * bi + 2, :, :],
                        start=first,
                        stop=stop,
                    )

            pool_tmps = []
            pool_banks = []
            for kh, kw, gh in pool_groups:
                if gh is None:
                    g0, ng, bks = 0, G, range(NBANK)
                else:
                    g0, ng, bks = 8 * gh, 8, range(4 * gh, 4 * gh + 4)
                gt = gpool.tile(
                    [PART, ng, NY, W], bf, name=f"gt{ng}", tag=f"gtmp{ng}"
                )
                nc.gpsimd.tensor_mul(gt[:], src_ap(kh, kw, g0, ng), w_ap(kh, kw, ng))
                pool_tmps.append(gt)
                pool_banks.append(bks)

            gi = 0

            def after_group(gi):
                if gi in pool_mm_pos:
                    i = pool_mm_pos[gi]
                    emit_mms(pool_tmps[i], False, banks=pool_banks[i])

            for kh, kw, gh in dve_halves:
                tmp = hpool.tile([PART, 8, NY, W], bf, name="th8", tag="tmph8")
                nc.vector.tensor_mul(
                    tmp[:], src_ap(kh, kw, 8 * gh, 8), w_ap(kh, kw, 8)
                )
                emit_mms(tmp, first=(gi in (0, 2)), banks=range(4 * gh, 4 * gh + 4))
                after_group(gi)
                gi += 1

            for kh, kw in dve_fulls:
                tmp = tpool.tile([PART, G, NY, W], bf, tag="tmp")
                nc.vector.tensor_mul(tmp[:], src_ap(kh, kw), w_ap(kh, kw))
                emit_mms(tmp, first=False)
                after_group(gi)
                gi += 1

            for q in range(4):
                for hi, (kh, kw) in enumerate(tails):
                    tmp = hpool.tile([PART, 4, NY, W], bf, tag="tmph")
                    nc.vector.tensor_mul(
                        tmp[:], src_ap(kh, kw, 4 * q, 4), w_ap(kh, kw, 4)
                    )
                    emit_mms(tmp, False, banks=range(2 * q, 2 * q + 2),
                             stop=(hi == len(tails) - 1))
                    after_group(gi)
                    gi += 1
                for bi in range(2):
                    j = 2 * q + bi
                    dst = ost[q][:, bi * BANK : (bi + 1) * BANK]
                    if q == 3 and bi == 1:
                        nc.vector.tensor_copy(dst, acc[j][:])
                    else:
                        nc.scalar.copy(dst, acc[j][:])
                nc.sync.dma_start(
                    out_d[:, 2 * q * BANK : 2 * (q + 1) * BANK], ost[q][:]
                )

    _dedupe_ldweights(nc)
    nc.compile()
    return nc


def _dedupe_ldweights(nc):
    """All PE matmuls share one identity stationary; drop every InstLdweights
    after the first (same AP, no semaphore activity)."""
    first_repr = None
    removed = 0
    for blk in nc.main_func.blocks:
        keep = []
        for inst in blk.instructions:
            if type(inst).__name__ == "InstLdweights":
                si = inst.sync_info
                clean = si is None or (not si.on_wait and not si.on_update)
                r = repr(inst.ins[0])
                if first_repr is None:
                    first_repr = r
                elif clean and r == first_repr:
                    removed += 1
                    continue
            keep.append(inst)
        blk.instructions[:] = keep
    return removed


def _prep_core(x, w, b, h):
    """Host-side shard prep for one core: (cc,ysub)-partition layouts."""
    xpad = np.zeros((G, WC, H + 2 * PAD, XC), dtype=np.float32)
    xpad[:, :, PAD : PAD + H, PAD : PAD + W] = x[b].reshape(G, WC, H, W)
    base = h * HHALF + np.arange(NSUB) * NY  # padded-row window starts
    ridx = base[:, None] + np.arange(HALO)[None, :]  # [8, 10]
    # xa[cc, ysub, g, r, col]
    xa = xpad[:, :, ridx, :].transpose(1, 2, 0, 3, 4)  # [WC,8,G,10,70]
    xa = np.ascontiguousarray(xa).reshape(PART, G, HALO, XC)
    xb = np.zeros_like(xa)
    xb[..., 1:] = xa[..., :-1]
    wv = w[b, 0][:, :, base[:, None] + np.arange(NY), :]  # [WC,49,8,4,64]
    wr = np.ascontiguousarray(wv.transpose(0, 2, 1, 3, 4)).reshape(PART, -1)
    cast = lambda a: np.ascontiguousarray(a).astype(_BF16)
    return {
        "xaPA": cast(xa[:, :8, :PROWS].transpose(0, 2, 1, 3).reshape(PART, -1)),
        "xaPB": cast(xa[:, 8:, :PROWS].transpose(0, 2, 1, 3).reshape(PART, -1)),
        "xaQ": cast(xa[:, :, QROW0:].reshape(PART, -1)),
        "xbP": cast(xb[:, :, :PROWS].reshape(PART, -1)),
        "xbQ": cast(xb[:, :, QROW0:].reshape(PART, -1)),
        "wr": cast(wr.reshape(PART, KK, NY, W)),
        "ident": np.eye(PART, dtype=_BF16),
    }


def kernel(x: np.ndarray, weight: np.ndarray) -> np.ndarray:
    from concourse.bass_utils import run_bass_kernel_spmd

    if "nc" not in _cache:
        _cache["nc"] = _build()
    nc = _cache["nc"]

    in_maps = [_prep_core(x, weight, core // 2, core % 2) for core in range(NCORES)]
    res = run_bass_kernel_spmd(nc, in_maps, list(range(NCORES)))

    out = np.empty((B, C, H, W), dtype=np.float32)
    for core in range(NCORES):
        b, h = core // 2, core % 2
        o = res.results[core]["out"].reshape(WC, NSUB, G, NY, W)
        # out[b, g*16+cc, h*32+ysub*4+y4, x] = o[cc, ysub, g, y4, x]
        oc = o.transpose(2, 0, 1, 3, 4).reshape(C, HHALF, W)
        out[b, :, h * HHALF : (h + 1) * HHALF, :] = oc
    return out
